# revision 103
# baseline (speedup 1.0000x reference)
"""Trainium2 Bass kernel for BertSelfAttentionWithRelations (RAT-style).

Sharding: 8 cores = 4 batches x 2 query-row halves; each core runs full
12-head attention for its (batch, 512 query rows) slab.

Factorized relation handling: softmax(qk/8 + qrel[i,rel]/8 + mask) is
computed as E = exp(qk/8) with exp(mask) folded into the value rows, scaled
per relation bin by g = exp(qrel/8):
  u_r = (E (.) M_r)^T @ [v | 1] PV matmuls give both ctx partials and bin
  sums c_r via the ones column (all seven u_r live in one PSUM tile).
  ctx = [g_0*u_tot + sum_r (g_r-g_0)*u_r + prz@[rv_0; rv_r-rv_0]] / Z
with prz = [Z, g_r*c_r] and Z = acc ones column.

Bin masks are built on-device from the shipped relation tensor with 4x-mode
is_equal passes while the PE is busy with the input projections. The j axis
is rotated per core so the query rows are a view of hT and softmax stays
permutation-invariant.
"""

from contextlib import ExitStack

import numpy as np
import ml_dtypes

import concourse.mybir as mybir
import concourse.tile as tile
from concourse import bacc
from concourse.bass_utils import run_bass_kernel_spmd
from concourse.masks import make_identity

F32 = mybir.dt.float32
F32R = mybir.dt.float32r
BF16 = mybir.dt.bfloat16
AF = mybir.ActivationFunctionType
ALU = mybir.AluOpType

B, S, HS, H, D = 4, 1024, 768, 12, 64
SH = S // 2          # rows per core
NIT = SH // 128      # 4 i-tiles per core
NC_CH = HS // 128    # 6 channel chunks
NJT = S // 128       # 8 j-chunks
VW = 65              # v block width per head (64 + ones column)


def _build_nc():
    nc = bacc.Bacc("TRN2", target_bir_lowering=False, debug=False, num_devices=8)

    dt_in = dict(kind="ExternalInput")
    hT = nc.dram_tensor("hT", [128, NC_CH, S], BF16, **dt_in).ap()
    wq = nc.dram_tensor("wq", [128, NC_CH, HS], BF16, **dt_in).ap()
    wk = nc.dram_tensor("wk", [128, NC_CH, HS], BF16, **dt_in).ap()
    wv = nc.dram_tensor("wv", [128, NC_CH, HS], BF16, **dt_in).ap()
    bqc = nc.dram_tensor("bqc", [128, NC_CH], F32, **dt_in).ap()
    bkc = nc.dram_tensor("bkc", [128, NC_CH], F32, **dt_in).ap()
    bvrow = nc.dram_tensor("bvrow", [1, HS], F32R, **dt_in).ap()
    em = nc.dram_tensor("em", [128, NJT], F32, **dt_in).ap()
    emrep = nc.dram_tensor("emrep", [128, NJT, H], F32, **dt_in).ap()
    relTf = nc.dram_tensor("relTf", [128, NJT, SH], BF16, **dt_in).ap()
    rkT = nc.dram_tensor("rkT", [128, 8], BF16, **dt_in).ap()
    rvd = nc.dram_tensor("rvd", [7, D], F32R, **dt_in).ap()
    out = nc.dram_tensor("out", [128, NIT, HS], F32, kind="ExternalOutput").ap()

    with tile.TileContext(nc) as tc, ExitStack() as ctx:
        # ---- persistent pools -------------------------------------------
        persist = ctx.enter_context(tc.tile_pool(name="persist", bufs=1))
        qTs = persist.tile([128, NC_CH, SH], BF16, tag="qTs")
        kTs = persist.tile([128, NC_CH, S], BF16, tag="kTs")
        vs = persist.tile([128, NJT, H, VW], BF16, tag="vs")
        mskA_sb = persist.tile([128, 6, NJT, 256], BF16, tag="mskA")
        relT_sb = persist.tile([128, NJT, SH], BF16, tag="relT")
        out_sb = persist.tile([128, 2, HS], F32, tag="outsb")
        em_sb = persist.tile([128, NJT], F32, tag="em")
        bq_sb = persist.tile([128, NC_CH], F32, tag="bq")
        bk_sb = persist.tile([128, NC_CH], F32, tag="bk")
        bv_sb = persist.tile([1, HS], F32R, tag="bv")
        emr_sb = persist.tile([128, NJT, H], F32, tag="emr")
        rkT_sb = persist.tile([128, 8], BF16, tag="rkT")
        rvd_sb = persist.tile([7, D], F32R, tag="rvd")
        identf = persist.tile([128, 128], F32, tag="identf")
        ones1 = persist.tile([1, 256], F32R, tag="ones1")
        ones1f = persist.tile([1, 256], F32, tag="ones1f")

        make_identity(nc, identf[:])
        nc.gpsimd.memset(ones1f[:], 1.0)
        nc.vector.tensor_copy(ones1[:], ones1f[:])

        # steady-state pools first so the stage-A pools sit on the pool
        # stack's top and can be popped (LIFO) mid-kernel
        scps = ctx.enter_context(tc.tile_pool(name="scps", bufs=2, space="PSUM"))
        epool = ctx.enter_context(tc.tile_pool(name="ework", bufs=2))
        spool = ctx.enter_context(tc.tile_pool(name="small", bufs=4))
        ucpool = ctx.enter_context(tc.tile_pool(name="ucp", bufs=2))
        ups = ctx.enter_context(tc.tile_pool(name="ups", bufs=3, space="PSUM"))
        yps = ctx.enter_context(tc.tile_pool(name="yps", bufs=1, space="PSUM"))

        # ---- stage A tiles; DMA order = need order ----------------------
        saH_cm = tc.tile_pool(name="saH", bufs=1)
        saH = saH_cm.__enter__()
        wk_cm = tc.tile_pool(name="saWk", bufs=1)
        saWk = wk_cm.__enter__()
        wq_cm = tc.tile_pool(name="saWq", bufs=1)
        saWq = wq_cm.__enter__()
        wv_cm = tc.tile_pool(name="saWv", bufs=1)
        saWv = wv_cm.__enter__()
        hT_sb = saH.tile([128, NC_CH, S], BF16, tag="hT")
        hTq_sb = hT_sb[:, :, 0:SH]  # j-axis is rotated so cols 0:SH are q rows
        wq_sb = saWq.tile([128, NC_CH, HS], BF16, tag="wq")
        wk_sb = saWk.tile([128, NC_CH, HS], BF16, tag="wk")
        wv_sb = saWv.tile([128, NC_CH, HS], BF16, tag="wv")
        nc.sync.dma_start(wq_sb[:], wq[:])
        nc.sync.dma_start(hT_sb[:, :, 0:SH], hT[:, :, 0:SH])
        nc.sync.dma_start(wk_sb[:], wk[:])
        nc.sync.dma_start(bq_sb[:], bqc[:])
        nc.sync.dma_start(bk_sb[:], bkc[:])
        nc.sync.dma_start(relT_sb[:], relTf[:])
        nc.sync.dma_start(hT_sb[:, :, SH:S], hT[:, :, SH:S])
        nc.sync.dma_start(wv_sb[:], wv[:])
        nc.sync.dma_start(em_sb[:], em[:])
        nc.sync.dma_start(bv_sb[:], bvrow[:])
        nc.sync.dma_start(emr_sb[:], emrep[:])
        nc.sync.dma_start(rkT_sb[:], rkT[:])
        nc.sync.dma_start(rvd_sb[:], rvd[:])
        # ones columns carry exp(mask_j) so the mask needs no matmul
        nc.vector.tensor_copy(vs[:, :, :, 64], emr_sb[:])
        # first-half bin masks built on DVE (idle during projections)
        for r in range(1, 7):
            nc.vector.tensor_scalar(
                mskA_sb[:, r - 1], relT_sb[:, :, 0:256], float(r), None,
                ALU.is_equal,
            )

        def emit_qproj(m):
            ps = scps.tile([128, 1024], F32, tag="sc")
            for n in range(NC_CH):
                nc.tensor.matmul(
                    ps[:, 0:512], wq_sb[:, n, m * 128:(m + 1) * 128],
                    hTq_sb[:, n, :],
                    start=(n == 0), stop=(n == NC_CH - 1),
                )
            nc.scalar.activation(qTs[:, m, :], ps[:, 0:512], AF.Identity,
                                 bias=bq_sb[:, m:m + 1])

        def emit_kproj(m, jh):
            ps = scps.tile([128, 1024], F32, tag="sc")
            for n in range(NC_CH):
                nc.tensor.matmul(
                    ps[:, 0:512], wk_sb[:, n, m * 128:(m + 1) * 128],
                    hT_sb[:, n, jh * 512:(jh + 1) * 512],
                    start=(n == 0), stop=(n == NC_CH - 1),
                )
            nc.scalar.activation(
                kTs[:, m, jh * 512:(jh + 1) * 512], ps[:, 0:512], AF.Identity,
                bias=bk_sb[:, m:m + 1]
            )

        def emit_vproj(jt, half):
            # v natural [j, hd] + bias, scaled by exp(mask_j)
            ps = scps.tile([128, 1024], F32, tag="sc")
            for n in range(NC_CH):
                nc.tensor.matmul(
                    ps[:, 0:384], hT_sb[:, n, jt * 128:(jt + 1) * 128],
                    wv_sb[:, n, half * 384:(half + 1) * 384],
                    start=(n == 0), stop=False,
                )
            nc.tensor.matmul(
                ps[:, 0:384], ones1[:, 0:128],
                bv_sb[:, half * 384:(half + 1) * 384],
                start=False, stop=True,
            )
            h0 = half * 6
            nc.scalar.activation(
                vs[:, jt, h0:h0 + 6, 0:64],
                ps[:, 0:384], AF.Copy, scale=em_sb[:, jt:jt + 1],
            )

        for m in range(NC_CH):
            emit_qproj(m)
        emit_kproj(0, 0)
        emit_kproj(0, 1)

        msk_tiles = {0: mskA_sb, 1: None}

        def emit_front(itp, h, pool_light=False):
            """Scores, exp and masked copies for head h."""
            po = (h % 2) * 64
            mch = h // 2
            qT_p = qTs[po:po + 64, mch, itp * 256:(itp + 1) * 256]

            # transposed scores scT[j, i] = k.q, exp -> ET bf16
            ET = epool.tile([128, NJT, 256], BF16, tag="ET")
            for half in range(2):
                scp = scps.tile([128, 1024], F32, tag="sc")
                for c in range(4):
                    jc = half * 4 + c
                    nc.tensor.matmul(
                        scp[:, c * 256:(c + 1) * 256],
                        kTs[po:po + 64, mch, jc * 128:(jc + 1) * 128],
                        qT_p, start=True, stop=True,
                    )
                nc.scalar.activation(
                    ET[:, half * 4:(half + 1) * 4, :], scp[:], AF.Exp, scale=0.125
                )

            # per-bin masked copies of ET (pair-wide); Pool takes 1.5 bins per
            # head on average (its multiply runs ~3.7x slower than DVE's).
            # Pool copies go in two j-halves so the PV matmuls can start on
            # the first half while the second is still being masked.
            ErTs = []
            msk = msk_tiles[itp]
            for r in range(1, 7):
                ErT = epool.tile([128, NJT, 256], BF16, tag=f"ErT{r}")
                on_pool = (r == 6) or (r == 5 and h % 2 == 1 and not pool_light)
                if on_pool:
                    for jh in range(2):
                        js = slice(jh * 4, (jh + 1) * 4)
                        nc.gpsimd.tensor_tensor(
                            ErT[:, js, :], ET[:, js, :],
                            msk[:, r - 1, js, :], ALU.mult,
                        )
                elif r == 4 and not pool_light:
                    # odd heads: Pool helps with the tail eighth of bin 4
                    nc.vector.tensor_tensor(
                        ErT[:, 0:7, :], ET[:, 0:7, :],
                        msk[:, r - 1, 0:7, :], ALU.mult,
                    )
                    nc.gpsimd.tensor_tensor(
                        ErT[:, 7:8, :], ET[:, 7:8, :],
                        msk[:, r - 1, 7:8, :], ALU.mult,
                    )
                else:
                    nc.vector.tensor_tensor(
                        ErT[:], ET[:], msk[:, r - 1, :, :], ALU.mult,
                    )
                ErTs.append(ErT)
            return ET, ErTs

        def emit_pvs(itp, h, ET, ErTs):
            """qrel/g/dg and the 7 bin PV matmul groups for head h."""
            po = (h % 2) * 64
            mch = h // 2
            state = []
            for a in range(2):
                isl = slice(a * 128, (a + 1) * 128)
                qT_h = qTs[po:po + 64, mch, itp * 256 + a * 128:
                           itp * 256 + (a + 1) * 128]

                # u_all bank also hosts the qrel scratch (cols 456:464)
                u_all = ups.tile([128, 464], F32, tag="u")
                qrel = u_all[:, 456:464]
                nc.tensor.matmul(
                    qrel, qT_h, rkT_sb[po:po + 64, :], start=True, stop=True,
                )
                g = spool.tile([128, 8], F32, tag="g")
                nc.scalar.activation(g[:, 0:7], qrel[:, 0:7], AF.Exp, scale=0.125)
                dg = spool.tile([128, 6], F32, tag="dg")
                nc.vector.tensor_scalar(
                    dg[:], g[:, 1:7], g[:, 0:1], None, ALU.subtract
                )

                # all 7 bin PVs accumulate into one PSUM tile
                for r in range(7):
                    src = ET if r == 0 else ErTs[r - 1]
                    for jc in range(NJT):
                        nc.tensor.matmul(
                            u_all[:, r * VW:(r + 1) * VW], src[:, jc, isl],
                            vs[:, jc, h, :],
                            start=(jc == 0), stop=(jc == NJT - 1),
                        )
                state.append((u_all, g, dg))
            return state

        def emit_tail_dve(state):
            """Evacuate u_all to SBUF (Act), then acc-chain + prz + rz on
            DVE without the per-op PSUM access penalty."""
            out_state = []
            for u_all, g, dg in state:
                # evacuate in two chunks: the DVE-bin part first so the
                # acc-chain can start before the Pool bins' PVs land
                uc = ucpool.tile([128, 7 * VW], F32, tag="uc")
                nc.scalar.copy(uc[:, 0:5 * VW], u_all[:, 0:5 * VW])
                nc.scalar.copy(uc[:, 5 * VW:7 * VW], u_all[:, 5 * VW:7 * VW])
                # acc cols 0:65 = sum_r ghat_r u_r; col 64 = Z;
                # cols 65:71 = g_r*c_r so [Z | g_r c_r] transposes in one AP
                acc = spool.tile([128, VW + 7], F32, tag="acc")
                nc.vector.tensor_scalar(
                    acc[:, 0:VW], uc[:, 0:VW], g[:, 0:1], None, ALU.mult
                )
                for r in range(1, 7):
                    nc.vector.scalar_tensor_tensor(
                        acc[:, 0:VW], uc[:, r * VW:(r + 1) * VW],
                        dg[:, r - 1:r], acc[:, 0:VW],
                        op0=ALU.mult, op1=ALU.add,
                    )
                nc.vector.tensor_tensor(
                    acc[:, VW:VW + 6], uc[:, 2 * VW - 1:7 * VW:VW],
                    g[:, 1:7], ALU.mult
                )
                rz = spool.tile([128, 1], F32, tag="rz")
                nc.vector.reciprocal(rz[:], acc[:, VW - 1:VW])
                out_state.append((acc, rz))
            return out_state

        def emit_tail_trans(state2):
            """prz transposes + SBUF copies (deps are >=1 iteration old)."""
            out2 = []
            for acc, rz in state2:
                yt = yps.tile([128, 192], F32, tag="y")
                przT = yt[0:8, 0:128]
                nc.tensor.transpose(przT[0:7, :], acc[:, VW - 1:VW + 6], identf[:])
                przT_sb = spool.tile([8, 128], F32R, tag="przTs")
                nc.scalar.copy(przT_sb[0:7, :], przT[0:7, :])
                out2.append((acc, rz, yt, przT_sb))
            return out2

        def emit_tail_out(itp, h, state3):
            """rel-v matmuls and normalized output (PE/Act side)."""
            for a, (acc, rz, yt, przT_sb) in enumerate(state3):
                cxr = yt[:, 128:192]
                nc.tensor.matmul(
                    cxr, przT_sb[0:7, :], rvd_sb[:], start=True, stop=False,
                )
                nc.tensor.matmul(
                    cxr, identf[:], acc[:, 0:64],
                    start=False, stop=True, skip_group_check=True,
                )
                nc.scalar.activation(
                    out_sb[:, a, h * 64:(h + 1) * 64], cxr, AF.Copy,
                    scale=rz[:],
                )

        # Software pipeline, two heads deep: per iteration h emit
        #   tail_dve(h-1)   acc-chain          (DVE first: frees u_all(h-1)
        #                                       before copies(h+1) can block
        #                                       the in-order DVE queue)
        #   tail_trans(h-2) prz transpose      (PE, deps >=1 iteration old)
        #   front(h+1)      scores/exp/copies  (PE early + Act, DVE/Pool)
        #   pvs(h)          qrel + 7 PV groups (PE)
        #   tail_out(h-2)   rel-v matmuls/out  (PE tail + Act; przT_sb copy
        #                                       happens on Act during PVs(h))
        # so no engine's in-order queue blocks bulk work behind a fresh
        # cross-engine dependency, keeping the PE stream dense (p-state).
        last_itp = NIT // 2 - 1

        def emit_out_full(ou, ostate):
            emit_tail_out(ou[0], ou[1], ostate)
            if ou == (last_itp, 5):
                # final itp: ship the first 6 heads' columns early so the
                # last out-DMA only covers half the row
                for a in range(2):
                    it = ou[0] * 2 + a
                    nc.sync.dma_start(out[:, it, 0:384], out_sb[:, a, 0:384])
            elif ou[1] == H - 1:
                for a in range(2):
                    it = ou[0] * 2 + a
                    if ou[0] == last_itp:
                        nc.sync.dma_start(out[:, it, 384:HS],
                                          out_sb[:, a, 384:HS])
                    else:
                        nc.sync.dma_start(out[:, it, :], out_sb[:, a, :])

        units = [(itp, h) for itp in range(NIT // 2) for h in range(H)]
        NU = len(units)
        cur_front = emit_front(*units[0], pool_light=True)
        prev_pv = None    # (unit, pv_state) awaiting tail_dve
        prev_acc = None   # (unit, acc_state) awaiting tail_trans
        prev_trans = None  # (unit, trans_state) awaiting tail_out

        def stage_a_inject_front(u1):
            # k-projection chunks woven one jh-half per unit, landing just
            # before the front of the head that needs chunk m = h1 // 2
            itp1, h1 = units[u1]
            if itp1 == 0 and 1 <= h1 <= 10:
                m, jh = (h1 + 1) // 2, (h1 + 1) % 2
                if m >= 1:
                    emit_kproj(m, jh)

        def stage_a_inject_pv(u):
            nonlocal mskB_open
            if u == 0:
                for jt in range(NJT):
                    emit_vproj(jt, 0)
            elif u == 1:
                for jt in range(NJT // 2):
                    emit_vproj(jt, 1)
            elif u == 2:
                for jt in range(NJT // 2, NJT):
                    emit_vproj(jt, 1)
            elif u == 3 and not mskB_open:
                wv_cm.__exit__(None, None, None)
                wq_cm.__exit__(None, None, None)
                # v/q projections done; reuse wv+wq SBUF for the second-half
                # masks so their DMA can start early (hT/wk stay open)
                mskB_cm = tc.tile_pool(name="mskB", bufs=1)
                mb = mskB_cm.__enter__()
                late_pools.append(mskB_cm)
                mskB_sb = mb.tile([128, 6, NJT, 256], BF16, tag="mskB")
                for r in range(1, 7):
                    nc.vector.tensor_scalar(
                        mskB_sb[:, r - 1], relT_sb[:, :, 256:512], float(r),
                        None, ALU.is_equal,
                    )
                msk_tiles[1] = mskB_sb
                mskB_open = True

        mskB_open = False
        late_pools = []
        for u in range(NU):
            new_acc = None
            if prev_pv is not None:
                pu, pstate = prev_pv
                new_acc = (pu, emit_tail_dve(pstate))
            new_trans = None
            if prev_acc is not None:
                tu, tstate = prev_acc
                new_trans = (tu, emit_tail_trans(tstate))
            nxt_front = None
            if u + 1 < NU:
                stage_a_inject_front(u + 1)
                nxt_front = emit_front(*units[u + 1], pool_light=(u + 1 <= 2))
            stage_a_inject_pv(u)
            pv_state = emit_pvs(*units[u], *cur_front)
            if prev_trans is not None:
                ou, ostate = prev_trans
                emit_out_full(ou, ostate)
            prev_acc = new_acc
            prev_trans = new_trans
            prev_pv = (units[u], pv_state)
            cur_front = nxt_front
        # drain the pipeline
        pu, pstate = prev_pv
        last_acc = (pu, emit_tail_dve(pstate))
        for src in (prev_acc, last_acc):
            if src is None:
                continue
            tu, tstate = src
            tr = emit_tail_trans(tstate)
            if prev_trans is not None:
                ou, ostate = prev_trans
                emit_out_full(ou, ostate)
                prev_trans = None
            emit_out_full(tu, tr)

        # pop remaining stage-A-era pools in LIFO order
        for cm in reversed(late_pools):
            cm.__exit__(None, None, None)
        wk_cm.__exit__(None, None, None)
        saH_cm.__exit__(None, None, None)

    nc.compile()
    return nc


_NC_CACHE = []


def _get_nc():
    if not _NC_CACHE:
        _NC_CACHE.append(_build_nc())
    return _NC_CACHE[0]


def _marshal(hidden_states, attention_mask, relation, Wq, bq, Wk, bk, Wv, bv,
             rel_k_emb, rel_v_emb):
    f32 = np.float32
    hidden_states = np.asarray(hidden_states, f32)
    attention_mask = np.asarray(attention_mask, f32)
    relation = np.asarray(relation)
    Wq, Wk, Wv = (np.ascontiguousarray(np.asarray(w, f32)) for w in (Wq, Wk, Wv))
    bq, bk, bv = (np.asarray(x, f32) for x in (bq, bk, bv))
    rv = np.asarray(rel_v_emb, f32)
    rvd = rv.copy()
    rvd[1:] -= rvd[0:1]

    def wchunk(w):
        return np.ascontiguousarray(
            w.reshape(NC_CH, 128, HS).transpose(1, 0, 2).astype(ml_dtypes.bfloat16))

    shared = {
        "wq": wchunk(Wq), "wk": wchunk(Wk), "wv": wchunk(Wv),
        "bqc": np.ascontiguousarray(bq.reshape(NC_CH, 128).T),
        "bkc": np.ascontiguousarray(bk.reshape(NC_CH, 128).T),
        "bvrow": np.ascontiguousarray(bv.reshape(1, HS)),
        "rkT": np.ascontiguousarray(
            np.pad(np.tile(np.asarray(rel_k_emb, f32).T, (2, 1)),
                   ((0, 0), (0, 1))).astype(ml_dtypes.bfloat16)),
        "rvd": np.ascontiguousarray(rvd),
    }
    in_maps = []
    for core in range(8):
        b, ih = core // 2, core % 2
        i0 = ih * SH
        # rotate the j (key/value) axis by i0 so this core's query rows are
        # hT's first SH columns; softmax over j is permutation-invariant as
        # long as k/v/mask/relation columns are rotated consistently.
        jperm = (np.arange(S) + i0) % S
        hTm = np.ascontiguousarray(hidden_states[b].T[:, jperm])  # [HS, S]
        emj = np.exp(attention_mask[b, 0, 0].astype(f32))[jperm]  # [S]
        emc = np.ascontiguousarray(emj.reshape(NJT, 128).T)  # [128, NJT]
        relT = relation[b, i0:i0 + SH][:, jperm].T  # [S, SH]
        rtf = relT.reshape(NJT, 128, SH).transpose(1, 0, 2)
        m = dict(shared)
        m["hT"] = np.ascontiguousarray(
            hTm.reshape(NC_CH, 128, S).transpose(1, 0, 2).astype(ml_dtypes.bfloat16))
        m["em"] = emc
        m["emrep"] = np.ascontiguousarray(
            np.repeat(emc[:, :, None], H, axis=2))
        m["relTf"] = np.ascontiguousarray(rtf.astype(ml_dtypes.bfloat16))
        in_maps.append(m)
    return in_maps


def kernel(hidden_states, attention_mask, relation, Wq, bq, Wk, bk, Wv, bv,
           rel_k_emb, rel_v_emb, _trace=False, _tmpdir=None):
    nc = _get_nc()
    in_maps = _marshal(hidden_states, attention_mask, relation, Wq, bq, Wk, bk,
                       Wv, bv, rel_k_emb, rel_v_emb)
    kw = {}
    if _trace:
        kw = dict(trace=True, tmpdir=_tmpdir)
    res = run_bass_kernel_spmd(nc, in_maps, core_ids=list(range(8)), **kw)
    out = np.zeros((B, S, HS), np.float32)
    for core in range(8):
        b, ih = core // 2, core % 2
        o = res.results[core]["out"]  # [128, NIT, HS]
        out[b, ih * SH:(ih + 1) * SH] = o.transpose(1, 0, 2).reshape(SH, HS)
    if _trace:
        return out, res
    return out
